# revision 1
# baseline (speedup 1.0000x reference)
"""BlazeEar NMS detection kernel for 8 Trainium2 NeuronCores.

Pipeline (SPMD, anchor axis sharded 8 ways):
  per core: load its 512K raw scores in 4 chunks (4 DMA queues), per-chunk
  per-partition top-8 + indices (DVE max/max_index, overlapped with DMA)
  -> merge to per-partition top-8 with global indices (one-hot matmul-free
     merge) -> exact local threshold at the 33rd-largest candidate (gpsimd
     kth_largest) -> compact surviving global indices (gpsimd sparse_gather)
  -> one indirect DMA gathers each survivor's [score|box|anchor] row from a
     host-interleaved (SLAB, 9) table -> AllGather 32 rows per core
  -> every core sorts the 256-slot score pool with 13 max8 rounds (stable,
     so f32 score ties resolve by ascending global index, matching
     jax.lax.top_k) -> indirect-gather the sorted top-100 rows, decode,
     IoU matrix, greedy-NMS via Jacobi fixed point, confidence mask
  -> (100, 5) output, identical on every core.
"""

import sys

sys.path.insert(0, "/opt/trn_rl_repo")

import numpy as np

import concourse.bass as bass
import concourse.bacc as bacc
import concourse.mybir as mybir
from concourse.tile import TileContext

A = 4194304
NCORES = 8
SLAB = A // NCORES          # 524288
P = 128
F = SLAB // P               # 4096
NCH = 4
W = F // NCH                # 1024
K = 100
NROUNDS = 13                # 13 * 8 = 104 >= 100 extracted values
KPAD = NROUNDS * 8          # 104
LOCAL_K = 31                # kth_largest k_adj -> threshold = 33rd largest local cand
SLOTS = 32                  # compacted candidate slots shipped per core
GLOB = NCORES * SLOTS       # 256
NROW = 9                    # candidate row: [score, rb0..rb3, ax, ay, aw, ah]
NMS_ITERS = 4
INPUT_SIZE_INV = 1.0 / 128.0
CONF = 0.75
IOU = 0.3
NEG = -1e30

f32 = mybir.dt.float32
i32 = mybir.dt.int32
u32 = mybir.dt.uint32
Alu = mybir.AluOpType
Act = mybir.ActivationFunctionType


PERTURB = None


def _build_program(debug=False):
    pert = PERTURB
    nc = bacc.Bacc()

    scores = nc.declare_dram_parameter("scores", [P, F], f32, isOutput=False)
    rows9 = nc.declare_dram_parameter("rows9", [SLAB, NROW], f32, isOutput=False)
    row_base = nc.declare_dram_parameter("row_base", [P, 1], f32, isOutput=False)
    base16 = nc.declare_dram_parameter("base16", [16, 1], f32, isOutput=False)
    ut = nc.declare_dram_parameter("ut", [K, K], f32, isOutput=False)
    rampu = nc.declare_dram_parameter("rampu", [16, SLOTS // 16], f32, isOutput=False)
    j32 = nc.declare_dram_parameter("j32", [P, 8 * NCH], f32, isOutput=False)
    out = nc.declare_dram_parameter("out", [K, 5], f32, isOutput=True)
    dummy = (nc.declare_dram_parameter("pdummy", [16, 64], f32, isOutput=True)
             if pert else None)

    dbg = {}
    if debug:
        for name, shape, dt in [
            ("d_v8", [P, 8], f32), ("d_g8", [P, 8], f32), ("d_kth", [1, 2], f32),
            ("d_ccin", [SLOTS, NROW], f32), ("d_ccout", [GLOB, NROW], f32),
            ("d_Va0", [1, GLOB], f32), ("d_SI", [1, KPAD], u32),
            ("d_g9", [K, NROW], f32), ("d_K6", [K, 6], f32), ("d_keep", [K, 1], f32),
        ]:
            dbg[name] = nc.declare_dram_parameter(name, shape, dt, isOutput=True)

    # internal DRAM
    gdram_b = nc.dram_tensor("gdram_b", [P * 8], f32)   # Gm bounce (128x8)
    cc_in = nc.dram_tensor("cc_in", [SLOTS, NROW], f32)
    cc_out = nc.dram_tensor("cc_out", [GLOB, NROW], f32, addr_space="Shared")
    si_dram = nc.dram_tensor("si_dram", [KPAD], u32)
    k6_dram = nc.dram_tensor("k6_dram", [6, K], f32)

    with TileContext(nc) as tc:
        with (
            tc.tile_pool(name="big", bufs=1) as bigp,
            tc.tile_pool(name="small", bufs=1) as sp,
            tc.tile_pool(name="psum", bufs=2, space="PSUM") as pp,
        ):
            # ---- stage A: chunked load + per-chunk top-8 and indices ----
            S = bigp.tile([P, F], f32)
            V32 = sp.tile([P, 8 * NCH], f32)
            Gmap = sp.tile([P, 8 * NCH], f32)   # global index of each candidate
            I32 = sp.tile([P, 8 * NCH], u32)
            rb = sp.tile([P, 1], f32)
            nc.sync.dma_start(out=rb[:], in_=row_base[:])
            dma_engines = [nc.sync, nc.scalar, nc.sync, nc.scalar]
            for ci in range(NCH):
                eng = dma_engines[ci % len(dma_engines)]
                eng.dma_start(out=S[:, ci * W:(ci + 1) * W],
                              in_=scores[:, ci * W:(ci + 1) * W])
                nc.vector.max(out=V32[:, ci * 8:(ci + 1) * 8],
                              in_=S[:, ci * W:(ci + 1) * W])
                nc.vector.max_index(out=I32[:, ci * 8:(ci + 1) * 8],
                                    in_max=V32[:, ci * 8:(ci + 1) * 8],
                                    in_values=S[:, ci * W:(ci + 1) * W])
                nc.vector.tensor_copy(out=Gmap[:, ci * 8:(ci + 1) * 8],
                                      in_=I32[:, ci * 8:(ci + 1) * 8])
                nc.vector.tensor_scalar(Gmap[:, ci * 8:(ci + 1) * 8],
                                        Gmap[:, ci * 8:(ci + 1) * 8],
                                        rb[:, 0:1], float(ci * W),
                                        op0=Alu.add, op1=Alu.add)

            if pert == "mi2":
                I32x = sp.tile([P, 8 * NCH], u32)
                for ci in range(NCH):
                    nc.vector.max_index(out=I32x[:, ci * 8:(ci + 1) * 8],
                                        in_max=V32[:, ci * 8:(ci + 1) * 8],
                                        in_values=S[:, ci * W:(ci + 1) * W])
                nc.sync.dma_start(out=dummy[8:9, 0:32], in_=I32x[0:1, :].bitcast(f32))
            # merge: row top-8 values + their global indices via one-hot select
            V8 = sp.tile([P, 8], f32)
            nc.vector.max(out=V8[:], in_=V32[:])
            pos8 = sp.tile([P, 8], u32)
            nc.vector.max_index(out=pos8[:], in_max=V8[:], in_values=V32[:])
            pos8f = sp.tile([P, 8], f32)
            nc.vector.tensor_copy(out=pos8f[:], in_=pos8[:])
            j32t = sp.tile([P, 8 * NCH], f32)
            nc.sync.dma_start(out=j32t[:], in_=j32[:, :])
            oh = sp.tile([P, 8 * 8 * NCH], f32)
            oh3 = oh[:].rearrange("p (a b) -> p a b", a=8)
            nc.vector.tensor_tensor(
                oh3, j32t[:].unsqueeze(1).to_broadcast([P, 8, 8 * NCH]),
                pos8f[:].unsqueeze(2).to_broadcast([P, 8, 8 * NCH]),
                op=Alu.is_equal)
            G8 = sp.tile([P, 8], f32)
            nc.vector.tensor_tensor(
                oh3, oh3, Gmap[:].unsqueeze(1).to_broadcast([P, 8, 8 * NCH]),
                op=Alu.mult)
            nc.vector.tensor_reduce(G8[:].unsqueeze(2), oh3,
                                    axis=mybir.AxisListType.X, op=Alu.add)
            if debug:
                nc.sync.dma_start(out=dbg["d_v8"][:], in_=V8[:])
                nc.sync.dma_start(out=dbg["d_g8"][:], in_=G8[:])

            # ---- stage B: local threshold + compaction ----
            kth = sp.tile([1, 2], f32)
            nc.gpsimd.kth_largest(kth[:], V8[:], n_per_lane=8, k=64,
                                  quantile=1.0 - (LOCAL_K + 0.5) / (P * 8 - 1))
            if debug:
                nc.sync.dma_start(out=dbg["d_kth"][:], in_=kth[:])
            if pert == "kth3":
                for _x in range(2):
                    kthx = sp.tile([1, 2], f32, tag=f"kthx{_x}")
                    nc.gpsimd.kth_largest(kthx[:], V8[:], n_per_lane=8, k=64,
                                          quantile=1.0 - (LOCAL_K + 0.5) / (P * 8 - 1))
                    nc.sync.dma_start(out=dummy[_x, 0:2].unsqueeze(0), in_=kthx[:])
            tb = sp.tile([P, 1], f32)
            nc.gpsimd.partition_broadcast(tb[:], kth[0:1, 1:2])

            m = sp.tile([P, 8], f32)
            nc.vector.tensor_scalar(m[:], V8[:], tb[:, 0:1], None, op0=Alu.is_gt)
            t2 = sp.tile([P, 8], f32)
            nc.vector.tensor_scalar_add(t2[:], m[:], -1.0)
            Gm = sp.tile([P, 8], f32)
            nc.vector.tensor_tensor(Gm[:], G8[:], m[:], op=Alu.mult)
            nc.vector.tensor_tensor(Gm[:], Gm[:], t2[:], op=Alu.add)

            nc.sync.dma_start(out=gdram_b[:], in_=Gm[:])
            sgin_g = sp.tile([16, 64], f32)
            # interleaved load: sparse_gather's scan order (f*16+p) must equal
            # the candidate order d[p*8+j] (ascending global index for ties)
            nc.sync.dma_start(out=sgin_g[:], in_=gdram_b[:].rearrange("(b a) -> a b", a=16))
            sgoG = sp.tile([16, SLOTS // 16], f32)
            nfG = sp.tile([1, 1], u32)
            nc.gpsimd.sparse_gather(sgoG[:], sgin_g[:], num_found=nfG[:])

            if pert == "sg3":
                for _x in range(2):
                    sgx = sp.tile([16, SLOTS // 16], f32, tag=f"sgx{_x}")
                    nfx = sp.tile([1, 1], u32, tag=f"nfx{_x}")
                    nc.gpsimd.sparse_gather(sgx[:], sgin_g[:], num_found=nfx[:])
                    nc.sync.dma_start(out=dummy[2 + _x, 0:SLOTS // 16].unsqueeze(0),
                                      in_=sgx[0:1, :])
            # HW sparse_gather leaves garbage past num_found; mask tail to -1
            nfb = sp.tile([16, 1], u32)
            nc.gpsimd.partition_broadcast(nfb[:], nfG[0:1, 0:1])
            nfbf = sp.tile([16, 1], f32)
            nc.vector.tensor_copy(out=nfbf[:], in_=nfb[:])
            rampt = sp.tile([16, SLOTS // 16], f32)
            nc.sync.dma_start(out=rampt[:], in_=rampu[:])
            msk = sp.tile([16, SLOTS // 16], u32)
            nc.vector.tensor_scalar(msk[:], rampt[:], nfbf[:, 0:1], None, op0=Alu.is_lt)
            neg1 = sp.tile([16, SLOTS // 16], f32)
            nc.vector.memset(neg1[:], -1.0)
            sgoGc = sp.tile([16, SLOTS // 16], f32)
            nc.vector.select(sgoGc[:], msk[:], sgoG[:], neg1[:])

            # local row ids; slot s = f*16+p, so column f holds slots
            # [f*16, (f+1)*16) in partition order -> two (16,1)-offset gathers
            b16 = sp.tile([16, 1], f32)
            nc.sync.dma_start(out=b16[:], in_=base16[:])
            li = sp.tile([16, SLOTS // 16], f32)
            nc.vector.tensor_scalar(li[:], sgoGc[:], b16[:, 0:1], None, op0=Alu.subtract)
            negm = sp.tile([16, SLOTS // 16], f32)
            nc.vector.tensor_scalar(negm[:], li[:], 0.0, None, op0=Alu.is_lt)
            nc.vector.tensor_scalar_mul(negm[:], negm[:], 8000000.0)
            nc.vector.tensor_tensor(li[:], li[:], negm[:], op=Alu.add)
            lii = sp.tile([16, SLOTS // 16], i32)
            nc.vector.tensor_copy(out=lii[:], in_=li[:])

            for h in range(SLOTS // 16):
                R9h = sp.tile([16, NROW], f32, tag=f"R9_{h}")
                nc.vector.memset(R9h[:], -1.0)
                nc.gpsimd.indirect_dma_start(
                    out=R9h[:], out_offset=None, in_=rows9[:, :],
                    in_offset=bass.IndirectOffsetOnAxis(ap=lii[:, h:h + 1], axis=0),
                    bounds_check=SLAB - 1, oob_is_err=False,
                )
                nc.sync.dma_start(out=cc_in[h * 16:(h + 1) * 16, :], in_=R9h[:])
            if debug:
                nc.sync.dma_start(out=dbg["d_ccin"][:], in_=cc_in[:, :])

            if pert == "ind3":
                for _x in range(2):
                    R9x = sp.tile([16, NROW], f32, tag=f"R9x{_x}")
                    nc.vector.memset(R9x[:], -1.0)
                    nc.gpsimd.indirect_dma_start(
                        out=R9x[:], out_offset=None, in_=rows9[:, :],
                        in_offset=bass.IndirectOffsetOnAxis(ap=lii[:, 0:1], axis=0),
                        bounds_check=SLAB - 1, oob_is_err=False,
                    )
                    nc.sync.dma_start(out=dummy[4 + _x, 0:NROW].unsqueeze(0),
                                      in_=R9x[0:1, :])
            # ---- stage C: AllGather + global sort ----
            nc.gpsimd.collective_compute(
                "AllGather", Alu.bypass,
                replica_groups=[list(range(NCORES))],
                ins=[cc_in[:, :]], outs=[cc_out[:, :]],
            )
            if pert == "cc2":
                cc_out2 = nc.dram_tensor("cc_out2", [GLOB, NROW], f32, addr_space="Shared")
                nc.gpsimd.collective_compute(
                    "AllGather", Alu.bypass,
                    replica_groups=[list(range(NCORES))],
                    ins=[cc_in[:, :]], outs=[cc_out2[:, :]],
                )
                nc.sync.dma_start(out=dummy[6, 0:NROW].unsqueeze(0), in_=cc_out2[0:1, :])
            Va = sp.tile([1, GLOB], f32)
            nc.sync.dma_start(out=Va[:], in_=cc_out[:, 0].unsqueeze(0))
            if debug:
                nc.sync.dma_start(out=dbg["d_ccout"][:], in_=cc_out[:, :])
                nc.sync.dma_start(out=dbg["d_Va0"][:], in_=Va[:])

            for r in range(NROUNDS):
                m8 = sp.tile([1, 8], f32, tag="m8")
                i8 = sp.tile([1, 8], u32, tag="i8")
                nc.vector.max(out=m8[:], in_=Va[:])
                nc.vector.max_index(out=i8[:], in_max=m8[:], in_values=Va[:])
                nc.vector.match_replace(out=Va[:], in_to_replace=m8[:],
                                        in_values=Va[:], imm_value=NEG)
                nc.sync.dma_start(out=si_dram[r * 8:(r + 1) * 8], in_=i8[:])
            if debug:
                d_si = sp.tile([1, KPAD], u32)
                nc.sync.dma_start(out=d_si[:], in_=si_dram[:].unsqueeze(0))
                nc.sync.dma_start(out=dbg["d_SI"][:], in_=d_si[:])

            if pert == "sort2":
                for r in range(NROUNDS):
                    m8x = sp.tile([1, 8], f32, tag="m8x")
                    i8x = sp.tile([1, 8], u32, tag="i8x")
                    nc.vector.max(out=m8x[:], in_=Va[:])
                    nc.vector.max_index(out=i8x[:], in_max=m8x[:], in_values=Va[:])
                    nc.vector.match_replace(out=Va[:], in_to_replace=m8x[:],
                                            in_values=Va[:], imm_value=NEG)
                nc.sync.dma_start(out=dummy[7, 0:8].unsqueeze(0), in_=m8x[:])
            # gather the winning rows in sorted order
            sic = sp.tile([KPAD, 1], u32)
            nc.sync.dma_start(out=sic[:], in_=si_dram[:].unsqueeze(1))
            g9 = sp.tile([K, NROW], f32)
            nc.gpsimd.indirect_dma_start(
                out=g9[:], out_offset=None,
                in_=cc_out[:, :],
                in_offset=bass.IndirectOffsetOnAxis(ap=sic[0:K, 0:1], axis=0),
                bounds_check=GLOB - 1, oob_is_err=False,
            )
            if debug:
                nc.sync.dma_start(out=dbg["d_g9"][:], in_=g9[:])

            # ---- stage D: decode in column layout (100 partitions) ----
            rb0, rb1 = g9[:, 1:2], g9[:, 2:3]
            rb2, rb3 = g9[:, 3:4], g9[:, 4:5]
            ax, ay, aw, ah = g9[:, 5:6], g9[:, 6:7], g9[:, 7:8], g9[:, 8:9]

            def tt(o, a, b, op):
                nc.vector.tensor_tensor(o, a, b, op=op)

            K6 = sp.tile([K, 6], f32)   # [y1, x1, y2, x2, area, score]
            xc = sp.tile([K, 1], f32)
            nc.vector.tensor_scalar_mul(xc[:], rb0, INPUT_SIZE_INV)
            tt(xc[:], xc[:], aw, Alu.mult)
            tt(xc[:], xc[:], ax, Alu.add)
            yc = sp.tile([K, 1], f32)
            nc.vector.tensor_scalar_mul(yc[:], rb1, INPUT_SIZE_INV)
            tt(yc[:], yc[:], ah, Alu.mult)
            tt(yc[:], yc[:], ay, Alu.add)
            wh = sp.tile([K, 1], f32)
            nc.vector.tensor_scalar_mul(wh[:], rb2, INPUT_SIZE_INV)
            tt(wh[:], wh[:], aw, Alu.mult)
            nc.vector.tensor_scalar_mul(wh[:], wh[:], 0.5)
            hh = sp.tile([K, 1], f32)
            nc.vector.tensor_scalar_mul(hh[:], rb3, INPUT_SIZE_INV)
            tt(hh[:], hh[:], ah, Alu.mult)
            nc.vector.tensor_scalar_mul(hh[:], hh[:], 0.5)

            ymin0 = sp.tile([K, 1], f32)
            ymax0 = sp.tile([K, 1], f32)
            xmin0 = sp.tile([K, 1], f32)
            xmax0 = sp.tile([K, 1], f32)
            tt(ymin0[:], yc[:], hh[:], Alu.subtract)
            tt(ymax0[:], yc[:], hh[:], Alu.add)
            tt(xmin0[:], xc[:], wh[:], Alu.subtract)
            tt(xmax0[:], xc[:], wh[:], Alu.add)
            tt(K6[:, 0:1], ymin0[:], ymax0[:], Alu.min)
            tt(K6[:, 2:3], ymin0[:], ymax0[:], Alu.max)
            tt(K6[:, 1:2], xmin0[:], xmax0[:], Alu.min)
            tt(K6[:, 3:4], xmin0[:], xmax0[:], Alu.max)
            dxr = sp.tile([K, 1], f32)
            tt(dxr[:], K6[:, 3:4], K6[:, 1:2], Alu.subtract)
            tt(K6[:, 4:5], K6[:, 2:3], K6[:, 0:1], Alu.subtract)
            tt(K6[:, 4:5], K6[:, 4:5], dxr[:], Alu.mult)
            scl = sp.tile([K, 1], f32)
            nc.vector.tensor_scalar_min(scl[:], g9[:, 0:1], 100.0)
            nc.vector.tensor_scalar_max(scl[:], scl[:], -100.0)
            nc.scalar.activation(K6[:, 5:6], scl[:], Act.Sigmoid)
            if debug:
                nc.sync.dma_start(out=dbg["d_K6"][:], in_=K6[:])

            # one DMA: first five K6 columns -> k6_dram rows (transposed store)
            nc.sync.dma_start(out=k6_dram[0:5, :].rearrange("j p -> p j"),
                              in_=K6[:, 0:5])

            # ---- stage E: NMS ----
            y1c, x1c, y2c, x2c = K6[:, 0:1], K6[:, 1:2], K6[:, 2:3], K6[:, 3:4]
            areac, scorec = K6[:, 4:5], K6[:, 5:6]
            BT = bigp.tile([K, 5 * K], f32, tag="BT")
            BT3 = BT[:].rearrange("p (a b) -> p a b", a=5)
            nc.sync.dma_start(
                out=BT3, in_=k6_dram[0:5, :].unsqueeze(0).to_broadcast([K, 5, K]))
            By1 = BT[:, 0 * K:1 * K]
            Bx1 = BT[:, 1 * K:2 * K]
            By2 = BT[:, 2 * K:3 * K]
            Bx2 = BT[:, 3 * K:4 * K]
            Bar = BT[:, 4 * K:5 * K]

            xx1 = bigp.tile([K, K], f32, tag="xx1")
            nc.vector.tensor_scalar(xx1[:], Bx1, x1c, None, op0=Alu.max)
            dx = bigp.tile([K, K], f32, tag="dx")
            nc.vector.scalar_tensor_tensor(dx[:], Bx2, x2c, xx1[:],
                                           op0=Alu.min, op1=Alu.subtract)
            nc.vector.tensor_scalar_max(dx[:], dx[:], 0.0)
            yy1 = bigp.tile([K, K], f32, tag="yy1")
            nc.vector.tensor_scalar(yy1[:], By1, y1c, None, op0=Alu.max)
            dy = bigp.tile([K, K], f32, tag="dy")
            nc.vector.scalar_tensor_tensor(dy[:], By2, y2c, yy1[:],
                                           op0=Alu.min, op1=Alu.subtract)
            nc.vector.tensor_scalar_max(dy[:], dy[:], 0.0)
            inter = bigp.tile([K, K], f32, tag="inter")
            tt(inter[:], dx[:], dy[:], Alu.mult)
            un = bigp.tile([K, K], f32, tag="un")
            nc.vector.scalar_tensor_tensor(un[:], Bar, areac, inter[:],
                                           op0=Alu.add, op1=Alu.subtract)
            nc.vector.tensor_scalar_max(un[:], un[:], 1e-9)
            M = bigp.tile([K, K], f32, tag="M")
            nc.vector.scalar_tensor_tensor(M[:], un[:], IOU, inter[:],
                                           op0=Alu.mult, op1=Alu.is_lt)
            UT = bigp.tile([K, K], f32, tag="UT")
            nc.sync.dma_start(out=UT[:], in_=ut[:, :])
            tt(M[:], M[:], UT[:], Alu.mult)

            keep = sp.tile([K, 1], f32)
            nc.vector.memset(keep[:], 1.0)
            for _ in range(NMS_ITERS):
                kv = pp.tile([K, 1], f32, tag="kv")
                nc.tensor.matmul(kv[:], M[:], keep[:])
                nc.vector.tensor_scalar(keep[:], kv[:], 0.5, None, op0=Alu.is_lt)
            cm = sp.tile([K, 1], f32)
            nc.vector.tensor_scalar(cm[:], scorec, CONF, None, op0=Alu.is_ge)
            tt(keep[:], keep[:], cm[:], Alu.mult)
            if debug:
                nc.sync.dma_start(out=dbg["d_keep"][:], in_=keep[:])

            O = sp.tile([K, 5], f32)
            nc.vector.tensor_scalar(O[:, 0:4], K6[:, 0:4], keep[:, 0:1], None, op0=Alu.mult)
            nc.vector.tensor_scalar(O[:, 4:5], scorec, keep[:, 0:1], None, op0=Alu.mult)
            nc.sync.dma_start(out=out[:], in_=O[:])

    nc.finalize()
    return nc


_NC_CACHE = None


def _get_nc():
    global _NC_CACHE
    if _NC_CACHE is None:
        _NC_CACHE = _build_program()
    return _NC_CACHE


def _make_in_maps(raw_boxes, raw_scores, anchors):
    raw_boxes = np.asarray(raw_boxes)
    raw_scores = np.asarray(raw_scores)
    anchors = np.asarray(anchors)
    ut_np = np.triu(np.ones((K, K), np.float32), k=1)
    ramp_np = np.arange(SLOTS, dtype=np.float32).reshape(SLOTS // 16, 16).T.copy()
    j32_np = np.broadcast_to(np.arange(8 * NCH, dtype=np.float32), (P, 8 * NCH)).copy()
    in_maps = []
    for c in range(NCORES):
        s = slice(c * SLAB, (c + 1) * SLAB)
        rows9_np = np.concatenate(
            [raw_scores[0, s, 0:1], raw_boxes[0, s, 0:4], anchors[s]], axis=1)
        in_maps.append({
            "scores": np.ascontiguousarray(raw_scores[0, s, 0].reshape(P, F)),
            "rows9": np.ascontiguousarray(rows9_np),
            "row_base": (c * SLAB + np.arange(P, dtype=np.float32) * F).reshape(P, 1),
            "base16": np.full((16, 1), c * SLAB, np.float32),
            "ut": ut_np,
            "rampu": ramp_np,
            "j32": j32_np,
        })
    return in_maps


def kernel(raw_boxes, raw_scores, anchors):
    from concourse.bass_utils import run_bass_kernel_spmd
    nc = _get_nc()
    in_maps = _make_in_maps(raw_boxes, raw_scores, anchors)
    res = run_bass_kernel_spmd(nc, in_maps, list(range(NCORES)))
    return np.asarray(res.results[0]["out"], dtype=np.float32)

